# revision 26
# baseline (speedup 1.0000x reference)
# Trainium2 Bass kernel for the MEGNet edge model:
#   out = relu(concat([src, dest, edge_attr, u[batch]], 1) @ W1 + b1) @ W2 + b2
#
# Strategy (8 NeuronCores, SPMD, edges sharded contiguously):
#  * All tensors are shipped in a transposed, feature-major layout [128, E_pad]
#    so the PE array contracts over features with no on-chip transposes; the
#    host transposes shards on the way in and the output on the way out.
#  * comb @ W1 decomposes into src@W1a + dest@W1b + edge_attr@W1c +
#    u[batch]@W1d.  The u[batch] term plus b1 folds into a per-group table
#    z = u @ W1d + b1 [G, 128]; batch is sorted, so each 512-edge tile spans
#    only a few consecutive groups and z[batch] is applied with one small
#    matmul per tile (lhsT = the k_s candidate z-rows, rhs = a one-hot
#    selection matrix built on the host).
#  * Traffic is the roofline: dest/edge_attr/output ship as bf16, src and the
#    one-hot sel matrix as fp8e4 (sel is 0/1, exact in fp8).  Total rel err
#    ~1.5e-2 vs the 2e-2 gate (validated numerically against the reference).
#  * The W2 matmuls run one tile-pair behind the accumulation matmuls so the
#    PE never stalls waiting for the ReLU; weights/zws/sel for the first
#    chunks load via the hardware-DGE queues (the gpsimd software queue takes
#    ~10us to produce its first descriptor batch).
import os
import numpy as np

N_CORES = 8
P = 128      # feature dim == SBUF partitions
TILE = 512   # edges per matmul tile (one PSUM bank of fp32)
CH = 14      # max matmul tiles per DMA chunk (pool slot size)
# chunk sizes: smaller leading chunks let compute start sooner, and a small
# final chunk keeps the output-DMA drain tail short
CHUNK_SIZES = [6, 6, 14, 14, 14, 14, 14, 14, 2]        # sums to 98 tiles
GRP = 6      # tiles per PE accumulation group (6 p1 banks + 2 p2 banks)
OG = 4       # tiles per output DMA group

MM_DTYPE = os.environ.get("KERNEL_MM_DTYPE", "bf16")
SRC_FP8 = os.environ.get("KERNEL_SRC_FP8", "1") == "1"
OUT_BF16 = os.environ.get("KERNEL_OUT_BF16", "1") == "1"

_prog_cache = {}


def _np_dtypes():
    import ml_dtypes
    mm = {"f32": np.float32, "f32r": np.float32,
          "bf16": ml_dtypes.bfloat16}[MM_DTYPE]
    sdt = ml_dtypes.float8_e4m3 if SRC_FP8 else mm
    return mm, sdt


def _build_program(T, k_s):
    import concourse.bacc as bacc
    import concourse.tile as tile
    from concourse import mybir

    f32 = mybir.dt.float32
    mdt = {"f32": mybir.dt.float32, "f32r": mybir.dt.float32r,
           "bf16": mybir.dt.bfloat16}[MM_DTYPE]
    sdt = mybir.dt.float8e4 if SRC_FP8 else mdt
    odt = mybir.dt.bfloat16 if OUT_BF16 else f32
    Relu = mybir.ActivationFunctionType.Relu
    Epad = T * TILE

    nc = bacc.Bacc("TRN2", target_bir_lowering=False, debug=False,
                   num_devices=N_CORES)
    srcT = nc.dram_tensor("srcT", [P, Epad], sdt, kind="ExternalInput")
    destT = nc.dram_tensor("destT", [P, Epad], mdt, kind="ExternalInput")
    eaT = nc.dram_tensor("eaT", [P, Epad], mdt, kind="ExternalInput")
    wpkd = nc.dram_tensor("wpk", [P, 4 * P], mdt, kind="ExternalInput")
    b2d = nc.dram_tensor("b2c", [P, 1], f32, kind="ExternalInput")
    seld = nc.dram_tensor("sel", [k_s, Epad], sdt, kind="ExternalInput")
    zwd = nc.dram_tensor("zw", [k_s, T * P], mdt, kind="ExternalInput")
    outT = nc.dram_tensor("outT", [P, Epad], odt, kind="ExternalOutput")

    assert sum(CHUNK_SIZES) == T
    CW = CH * TILE  # max chunk width in edges (pool slot size)

    with tile.TileContext(nc) as tc:
        with (
            tc.tile_pool(name="const", bufs=1) as constp,
            tc.tile_pool(name="inp", bufs=3) as inp,
            tc.tile_pool(name="hp", bufs=14) as hp,
            tc.tile_pool(name="outp", bufs=6) as outp,
            tc.tile_pool(name="ps1", bufs=6, space="PSUM") as ps1,
            tc.tile_pool(name="ps2", bufs=2, space="PSUM") as ps2,
        ):
            wpk = constp.tile([P, 4 * P], mdt, tag="wpk", name="wpk")
            b2s = constp.tile([P, 1], f32, tag="b2s", name="b2s")
            b2w = constp.tile([P, 1], f32, tag="b2w", name="b2w")
            zws = constp.tile([k_s, T * P], mdt, tag="zws", name="zws")
            nc.sync.dma_start(wpk[:], wpkd[:])
            # dummy early transfer to spin up the gpsimd software-DGE queue
            # (it takes ~10us from first use to first delivered packet; the
            # output tiles that ride it are first ready at ~20us)
            nc.gpsimd.dma_start(b2w[:], b2d[:])
            w1a = wpk[:, 0:P]
            w1b = wpk[:, P:2 * P]
            w1c = wpk[:, 2 * P:3 * P]
            w2s = wpk[:, 3 * P:4 * P]

            # The PE stream works in GRP-tile accumulation groups: one
            # stationary load per weight per group feeds GRP consecutive
            # matmuls (long same-weight runs keep the PE array streaming),
            # using GRP p1 PSUM banks.  The W2 matmuls of group g flush one
            # at a time during group g+1's selection phase, so the PE never
            # waits on a ReLU.  Output tiles cover OG tiles each; their DMA
            # issues right after the vector add of their last tile.
            pend = []  # [(h_tile, tile_idx)]
            ot_cur = [None, 0]  # current output tile, its base tile idx

            def flush_one():
                if not pend:
                    return
                h, ti = pend.pop(0)
                p2 = ps2.tile([P, TILE], f32, tag="p2", name=f"p2_{ti}")
                nc.tensor.matmul(p2[:], w2s, h[:], start=True, stop=True)
                og = ti // OG
                o0 = og * OG  # first tile of the output group
                ow = (min(o0 + OG, T) - o0) * TILE
                if ot_cur[0] is None or ot_cur[1] != o0:
                    ot_cur[0] = outp.tile([P, OG * TILE], odt, tag="o",
                                          name=f"ot{og}")
                    ot_cur[1] = o0
                ot = ot_cur[0]
                ocs = slice((ti - o0) * TILE, (ti - o0 + 1) * TILE)
                nc.vector.tensor_scalar_add(ot[:, ocs], p2[:], b2s[:])
                if ti == min(o0 + OG, T) - 1:
                    # last tile of this output group: ship it.  Mid-kernel
                    # outputs ride gpsimd's software-DGE queue (keeps trigger
                    # instructions off sync/scalar); the final groups take
                    # the lower-latency sync queue to shorten the drain.
                    eng = nc.sync if ti >= T - 9 else nc.gpsimd
                    eng.dma_start(
                        outT[:, o0 * TILE:o0 * TILE + ow], ot[:, :ow])

            def emit_group(tiles):
                p1s = [ps1.tile([P, TILE], f32, tag="p1", name=f"p1_{ti}")
                       for ti, _, _ in tiles]
                for wi, w in enumerate((w1a, w1b, w1c)):
                    for i, (ti, bufs, cs) in enumerate(tiles):
                        nc.tensor.matmul(p1s[i][:], w, bufs[wi][:, cs],
                                         start=(wi == 0), stop=False)
                for i, (ti, bufs, cs) in enumerate(tiles):
                    for j0 in range(0, k_s, P):
                        j1 = min(j0 + P, k_s)
                        nc.tensor.matmul(p1s[i][:],
                                         zws[j0:j1, ti * P:(ti + 1) * P],
                                         bufs[3][j0:j1, cs],
                                         start=False, stop=(j1 == k_s))
                    flush_one()
                    h = hp.tile([P, TILE], mdt, tag="h", name=f"h{ti}")
                    nc.scalar.activation(h[:], p1s[i][:], Relu)
                    pend.append((h, ti))

            t = 0
            tile_q = []
            for ci, csz in enumerate(CHUNK_SIZES):
                base = t * TILE
                cw = csz * TILE
                st = inp.tile([P, CW], sdt, tag="src", name=f"st{ci}")
                dt = inp.tile([P, CW], mdt, tag="dest", name=f"dt{ci}")
                et = inp.tile([P, CW], mdt, tag="ea", name=f"et{ci}")
                slt = inp.tile([k_s, CW], sdt, tag="sel", name=f"slt{ci}")
                # every input stream rides the sync hardware-DGE queue: sync
                # has no per-tile compute, so trigger instructions (~0.7us
                # each) never gate the ReLU/add pipeline on scalar/vector
                nc.sync.dma_start(st[:, :cw], srcT[:, base:base + cw])
                nc.sync.dma_start(dt[:, :cw], destT[:, base:base + cw])
                nc.sync.dma_start(et[:, :cw], eaT[:, base:base + cw])
                nc.sync.dma_start(slt[:, :cw], seld[:, base:base + cw])
                if ci == 0:  # constants queued behind chunk 0's streams
                    nc.sync.dma_start(zws[:], zwd[:])
                    nc.sync.dma_start(b2s[:], b2d[:])

                for tl in range(csz):
                    cs = slice(tl * TILE, (tl + 1) * TILE)
                    tile_q.append((t, (st, dt, et, slt), cs))
                    t += 1
                while len(tile_q) >= GRP:
                    emit_group(tile_q[:GRP])
                    del tile_q[:GRP]
            if tile_q:
                emit_group(tile_q)
            while pend:
                flush_one()

    nc.compile()
    return nc


def _get_program(T, k_s):
    key = (T, k_s)
    if key not in _prog_cache:
        _prog_cache[key] = _build_program(T, k_s)
    return _prog_cache[key]


def _install_profile_shim():
    """Optional: enable NTFF profiling under axon (KERNEL_PROFILE=1)."""
    import sys, types
    if "antenv.axon_hooks" not in sys.modules:
        mod = types.ModuleType("antenv.axon_hooks")
        mod._hook = None
        mod.set_axon_ntff_profile_hook = lambda h: setattr(mod, "_hook", h)
        mod.get_axon_ntff_profile_hook = lambda: mod._hook
        sys.modules["antenv.axon_hooks"] = mod
        try:
            import antenv
            antenv.axon_hooks = mod
        except ImportError:
            pass
        try:
            from trn_agent_boot.trn_boot import _ntff_profile_via_ctypes
            mod.set_axon_ntff_profile_hook(
                _ntff_profile_via_ctypes("/opt/axon/libaxon_pjrt.so"))
        except Exception:
            pass
    import concourse.bass_utils as bass_utils
    bass_utils.upload_artifacts = lambda tmpdir: tmpdir


def kernel(src, dest, edge_attr, u, batch, W1, b1, W2, b2):
    src = np.asarray(src, dtype=np.float32)
    dest = np.asarray(dest, dtype=np.float32)
    edge_attr = np.asarray(edge_attr, dtype=np.float32)
    u = np.asarray(u, dtype=np.float32)
    W1 = np.asarray(W1, dtype=np.float32)
    b1 = np.asarray(b1, dtype=np.float32)
    W2 = np.asarray(W2, dtype=np.float32)
    b2 = np.asarray(b2, dtype=np.float32)
    b = np.asarray(batch).astype(np.int64)

    E, D = src.shape
    G = u.shape[0]
    assert D == P and E % N_CORES == 0
    E0 = E // N_CORES
    CW = CH * TILE
    Epad = ((E0 + CW - 1) // CW) * CW
    T = Epad // TILE

    # Fold u[batch] @ W1d + b1 into a per-group table (tiny: G x D).
    z = (u @ W1[3 * D:4 * D] + b1).astype(np.float32)  # [G, D]

    # Per-core: tile-local group offsets for the z-selection matmul.
    g0s, js = [], []
    k_s = 1
    for c in range(N_CORES):
        bc = b[c * E0:(c + 1) * E0]
        bp = np.concatenate([bc, np.full(Epad - E0, bc[-1], dtype=np.int64)])
        per_tile = bp.reshape(T, TILE)
        g0 = per_tile.min(axis=1)                 # [T]
        j = bp - np.repeat(g0, TILE)              # [Epad], >= 0
        g0s.append(g0)
        js.append(j)
        k_s = max(k_s, int(j.max()) + 1)

    mmdt, sdt = _np_dtypes()
    in_maps = []
    wpk_in = np.concatenate(
        [W1[0 * D:1 * D], W1[1 * D:2 * D], W1[2 * D:3 * D], W2],
        axis=0).reshape(4, D, D).transpose(1, 0, 2).reshape(D, 4 * D)
    wpk_in = np.ascontiguousarray(wpk_in).astype(mmdt)
    b2_in = np.ascontiguousarray(b2.reshape(P, 1))
    for c in range(N_CORES):
        sl = slice(c * E0, (c + 1) * E0)

        def tr(x, dt):
            out = np.zeros((P, Epad), dtype=dt)
            out[:, :E0] = x[sl].T.astype(dt)
            return out

        selc = np.zeros((k_s, Epad), dtype=sdt)
        selc[js[c], np.arange(Epad)] = 1.0
        selc[:, E0:] = 0.0  # pad edges contribute nothing
        gidx = np.clip(g0s[c][:, None] + np.arange(k_s)[None, :], 0, G - 1)
        zwc = np.ascontiguousarray(
            z[gidx].transpose(1, 0, 2).reshape(k_s, T * P)).astype(mmdt)
        in_maps.append({
            "srcT": tr(src, sdt), "destT": tr(dest, mmdt),
            "eaT": tr(edge_attr, mmdt),
            "wpk": wpk_in, "b2c": b2_in,
            "sel": selc, "zw": zwc,
        })

    profile = os.environ.get("KERNEL_PROFILE", "") == "1"
    if profile:
        _install_profile_shim()

    nc = _get_program(T, k_s)
    from concourse.bass_utils import run_bass_kernel_spmd
    kwargs = {}
    if profile:
        kwargs["trace"] = True
        if os.environ.get("KERNEL_PROFILE_ALL", "") == "1":
            kwargs["trace_cores"] = list(range(N_CORES))
    res = run_bass_kernel_spmd(nc, in_maps, core_ids=list(range(N_CORES)),
                               **kwargs)
    if profile and res.exec_time_ns is not None:
        with open("/tmp/kernel_exec_ns.txt", "w") as f:
            f.write(str(res.exec_time_ns))
        print(f"HW exec time: {res.exec_time_ns} ns")

    out = np.empty((E, P), dtype=np.float32)
    for c in range(N_CORES):
        out[c * E0:(c + 1) * E0] = \
            res.results[c]["outT"][:, :E0].T.astype(np.float32)
    return out
